# revision 26
# baseline (speedup 1.0000x reference)
"""CTREmbedding Trainium2 kernel.

out[b,l,m,e] = interval-embedding interpolation:
    v  = (l < traj_length[b])                       in {0,1}
    ds = v ? mat2[traj_location[b,l]-1, m] : 0
    dt = vector[b,l]
    out = ds * S1[e] + C0[e] + v*Cv[e] + dt*Ct[e] + v*dt*Cvt[e]

Per (b,l) pair the [M,E] block is one K=20 fp32 matmul:
    lhsT [20,128] : rows 0..15 = dsT (dsT[j,p] = ds[16p+j]); rows 16..19 =
                    per-pair scalars (1, v, dt, v*dt) broadcast x128
    rhs  [20,800] : rows 0..15 = block-diag S1 (rhs[j, j*50+e] = S1[e])
                    rows 16..19 = C-basis vectors tiled 16x along free dim
    out  [128,800]: out[p, j*50+e] = value at m=16p+j  -> partition p owns
                    m in [16p,16p+16) = 3200 contiguous output bytes.

The dsT blocks are gathered on HOST (only 400 of 4096 mat2 rows are used;
3.2MB of host work) and shipped in the consts input — a device-side SWDGE
indirect-gather chain measured 70us serialized and did not overlap with
the output DMAs, doubling kernel time.

Sharding: 400 (b,l) pairs, 50 per core on 8 cores; each core writes a
contiguous [50, M*E] slice. Per-core roofline = 20.5MB HBM write ~ 60us;
the kernel is output-DMA bound. 6 rotating output buffers hide the ~2-3us
per-DMA completion latency that otherwise gates buffer reuse (measured
~40us of pipeline bubbles with 3 buffers).
"""

import numpy as np

B, L, M, E, NLOC = 4, 100, 2048, 50, 4096
EX_SU, EX_SL, EX_TU, EX_TL = 1000.0, 0.0, 86400.0, 0.0

N_CORES = 8
PAIRS = B * L                      # 400
PPC = PAIRS // N_CORES             # 50 pairs per core
JJ = 16                            # m-values per partition
PCH = M // JJ                      # 128 partitions
FREE = JJ * E                      # 800 floats per partition per pair
GROUP = 4                          # pairs per output DMA (4 * 400KB)
SIZES = [1, 3, 2] + [GROUP] * ((PPC - 6) // GROUP)   # leading small groups
LW = PPC * PCH                     # 6400: dsT+sigma region width
CW = LW + FREE                     # 7200: + rhs table columns

_cache = {}


def _build_bass(out_dt_name="float32", sizes=None, split=(512, 288),
                repeat=1, ap="inter", obufs=6, pbufs=4, group=GROUP,
                dmaq="sync", nmm=2):
    import concourse.tile as tile
    from concourse import bacc, mybir
    from concourse.tile import add_dep_helper

    f32 = mybir.dt.float32
    out_dt = getattr(mybir.dt, out_dt_name)
    osz = 4 if out_dt_name == "float32" else 2
    sizes = sizes or SIZES

    nc = bacc.Bacc("TRN2", target_bir_lowering=False, debug=False,
                   num_devices=N_CORES)
    # consts[0:16, 0:6400] = dsT; consts[16:20, 0:6400] = sigma;
    # consts[:, 6400:7200] = rhs table
    consts = nc.declare_dram_parameter("consts", [20, CW], f32,
                                       isOutput=False)
    # ap="inter": pair-major [PPC, M*E] with interleaved (p,q,r) DMA APs —
    # measured faster on HW than the partition-major "plain" layout.
    if ap == "inter":
        out = nc.declare_dram_parameter("out", [PPC, M * E], out_dt,
                                        isOutput=True)
    else:
        out = nc.declare_dram_parameter("out", [PCH, PPC * FREE], out_dt,
                                        isOutput=True)

    with tile.TileContext(nc) as tc:
        with (
            tc.tile_pool(name="const", bufs=1) as cpool,
            tc.tile_pool(name="outp", bufs=obufs) as opool,
            tc.tile_pool(name="psum", bufs=pbufs, space="PSUM") as ppool,
        ):
            lhs_sb = cpool.tile([20, CW], f32)
            # rhs cols first (tiny): first matmuls wait only on this plus
            # the leading chunk of the dsT/sigma load. dsT is split so
            # pair 0 starts after ~100KB lands, not the full 512KB.
            dma_rhs = nc.sync.dma_start(out=lhs_sb[:, LW:CW],
                                        in_=consts[:, LW:CW])
            DS0 = 8 * PCH            # first 8 pairs' columns
            dma_ds0 = nc.sync.dma_start(out=lhs_sb[:, 0:DS0],
                                        in_=consts[:, 0:DS0])
            nc.sync.dma_start(out=lhs_sb[:, DS0:LW],
                              in_=consts[:, DS0:LW])
            # warmup matmul absorbs the leading input-DMA waits on PE and
            # starts the HAM ramp before the first real pair arrives
            wps = ppool.tile([PCH, 200 if nmm == 4 else FREE], f32,
                             tag="ps")
            wmm = nc.tensor.matmul(
                out=wps[0:4, 0:4], lhsT=lhs_sb[0:20, CW - 8: CW - 4],
                rhs=lhs_sb[0:20, CW - 4: CW], start=True, stop=True,
            )
            add_dep_helper(wmm.ins, dma_ds0.ins, True, "absorb dsT wait")

            for _rep in range(repeat):
                _loop_body(nc, opool, ppool, lhs_sb, sizes, split,
                           out_dt, out, ap, group, dmaq, nmm)
    nc.compile()
    return nc


def _loop_body(nc, opool, ppool, lhs_sb, sizes, split, out_dt, out, ap,
               group, dmaq="sync", nmm=2):
    from concourse import mybir

    f32 = mybir.dt.float32
    s0, s1 = split
    i0 = 0
    for g, ng in enumerate(sizes):
        out_sb = opool.tile([PCH, group * FREE], out_dt, tag="out_sb")
        for q in range(ng):
            i = i0 + q
            lhsT = lhs_sb[0:20, i * PCH: (i + 1) * PCH]
            dst = out_sb[:, q * FREE: (q + 1) * FREE]
            if nmm == 4:
                # 4x200-col matmuls into single-bank PSUM tiles: 8 bufs
                # in flight for cross-matmul ILP
                for c4 in range(4):
                    ps = ppool.tile([PCH, 200], f32, tag="ps")
                    nc.tensor.matmul(
                        out=ps[:, :], lhsT=lhsT,
                        rhs=lhs_sb[0:20, LW + c4 * 200: LW + (c4 + 1) * 200],
                        start=True, stop=True,
                    )
                    c = (nc.vector.tensor_copy if c4 < 2 else nc.scalar.copy)
                    c(out=dst[:, c4 * 200: (c4 + 1) * 200], in_=ps[:, :])
            elif nmm == 1:
                # single 800-col matmul: halves PE instruction count and
                # per-pair lhsT stationary loads
                ps = ppool.tile([PCH, FREE], f32, tag="ps")
                nc.tensor.matmul(
                    out=ps[:, :], lhsT=lhsT,
                    rhs=lhs_sb[0:20, LW: CW],
                    start=True, stop=True,
                )
                nc.vector.tensor_copy(out=dst[:, 0:s0], in_=ps[:, 0:s0])
                nc.scalar.copy(out=dst[:, s0:FREE], in_=ps[:, s0:FREE])
            else:
                ps = ppool.tile([PCH, FREE], f32, tag="ps")
                nc.tensor.matmul(
                    out=ps[:, 0:512], lhsT=lhsT,
                    rhs=lhs_sb[0:20, LW: LW + 512],
                    start=True, stop=True,
                )
                nc.tensor.matmul(
                    out=ps[:, 512:FREE], lhsT=lhsT,
                    rhs=lhs_sb[0:20, LW + 512: CW],
                    start=True, stop=True,
                )
                if s0 > 0:
                    nc.vector.tensor_copy(out=dst[:, 0:s0],
                                          in_=ps[:, 0:s0])
                if s0 < FREE:
                    nc.scalar.copy(out=dst[:, s0:FREE],
                                   in_=ps[:, s0:FREE])
        eng = nc.sync if (dmaq == "sync" or g % 2 == 0) else nc.scalar
        if ap == "inter":
            dram_ap = out[i0: i0 + ng, :].rearrange("q (p r) -> p q r",
                                                    p=PCH)
            sb_ap = out_sb[:, 0: ng * FREE].rearrange("p (q r) -> p q r",
                                                      q=ng)
            eng.dma_start(out=dram_ap, in_=sb_ap)
        else:
            eng.dma_start(out=out[:, i0 * FREE: (i0 + ng) * FREE],
                          in_=out_sb[:, 0: ng * FREE])
        i0 += ng


def _host_prep(inputs):
    traj_location = np.asarray(inputs["traj_location"]).astype(np.int64)
    mat2 = np.asarray(inputs["mat2"], dtype=np.float32)
    vector = np.asarray(inputs["vector"], dtype=np.float32)
    traj_length = np.asarray(inputs["traj_length"]).astype(np.int64)
    emb_su = np.asarray(inputs["emb_su"], dtype=np.float32)
    emb_sl = np.asarray(inputs["emb_sl"], dtype=np.float32)
    emb_tu = np.asarray(inputs["emb_tu"], dtype=np.float32)
    emb_tl = np.asarray(inputs["emb_tl"], dtype=np.float32)

    # ---- host prep: O(B*L) scalars + a 3.2MB row gather ----
    valid = (np.arange(L)[None, :] < traj_length[:, None]).reshape(-1)  # [400]
    v = valid.astype(np.float32)
    dt = vector.reshape(-1)
    loc0 = (traj_location.reshape(-1) - 1).astype(np.int64)
    ds = np.where(valid[:, None], mat2[loc0], np.float32(0.0))  # [400, M]

    # rhs table [20, 800]
    S1 = (emb_su[1] - emb_sl[1]) / (EX_SU - EX_SL)
    C0 = emb_sl[0] + emb_tl[0]
    Cv = (emb_sl[1] + emb_tl[1]) - (emb_sl[0] + emb_tl[0])
    Ct = (emb_tu[0] - emb_tl[0]) / (EX_TU - EX_TL)
    Cvt = ((emb_tu[1] - emb_tl[1]) - (emb_tu[0] - emb_tl[0])) / (EX_TU - EX_TL)
    rhstab = np.zeros((20, FREE), np.float32)
    for j in range(JJ):
        rhstab[j, j * E: (j + 1) * E] = S1
    rhstab[16, :] = np.tile(C0, JJ)
    rhstab[17, :] = np.tile(Cv, JJ)
    rhstab[18, :] = np.tile(Ct, JJ)
    rhstab[19, :] = np.tile(Cvt, JJ)

    in_maps = []
    for c in range(N_CORES):
        sl = slice(c * PPC, (c + 1) * PPC)
        # dsT[j, i*128 + p] = ds[i, 16p + j]
        dsT = np.ascontiguousarray(
            ds[sl].reshape(PPC, PCH, JJ).transpose(2, 0, 1).reshape(JJ, LW)
        )
        sig = np.stack([np.ones(PPC, np.float32), v[sl], dt[sl],
                        v[sl] * dt[sl]])
        sigma = np.repeat(sig, PCH, axis=1).astype(np.float32)
        consts = np.empty((20, CW), np.float32)
        consts[0:JJ, 0:LW] = dsT
        consts[16:20, 0:LW] = sigma
        consts[:, LW:CW] = rhstab
        in_maps.append({"consts": consts})
    return in_maps


def kernel(**inputs):
    from concourse.bass_utils import run_bass_kernel_spmd

    in_maps = _host_prep(inputs)
    if "nc" not in _cache:
        _cache["nc"] = _build_bass()
    res = run_bass_kernel_spmd(_cache["nc"], in_maps,
                               core_ids=list(range(N_CORES)))
    parts = [np.asarray(res.results[c]["out"]).reshape(PPC, M, E)
             for c in range(N_CORES)]
    return np.concatenate(parts, axis=0).reshape(B, L, M, E).astype(np.float32)
